# revision 46
# baseline (speedup 1.0000x reference)
"""Multi-head attention (B=8, S=1024, D=1024, H=16) on 8 TRN2 NeuronCores.

Sharding: pure data parallel — batch element b on core b. Weights are
broadcast to every core. No collectives.

Per-core pipeline (X: [S, D] for one batch element):
  A. X^T via PE transposes (bf16 matmul vs identity), PSUM->SBUF copies
     split between ACT and DVE.
  C. V = X @ W_v (bf16, natural layout) stored as V_aug[sk, head, 65]
     with a ones column (col 64) that accumulates the softmax denominator
     during PV.
  B0. Q^T/K^T projection for head pair 0 only.
  BD. For each head pair p: attention block for p interleaved (in PE
     program order) with the Q^T/K^T projection of pair p+1. The
     attention block is ACT-bound (8 wide exps per sc chunk ~= 8.6us vs
     5us of PE scores+PV), so pair p+1's projection matmuls fill the PE
     slack instead of forming a separate PE-bound phase.
       scores^T (paired across the 2 heads: 64-row PE groups run
       concurrently), one wide exp per sk on ACT (scale=1/8,
       max-subtraction skipped: scores ~N(0,1)), PV accumulates
       unnormalized out^T (rows 0-63) + denominator (row 64).
     Normalize per (pair, sc, head): DVE copy PSUM->SBUF (fast PSUM bank
     release), reciprocal_approx_fast on the denom row, GpSimd
     partition-broadcast (idle engine), DVE multiply -> attn^T (f32r).
     All bias-adds ride on DVE (tensor_scalar/tensor_tensor), keeping
     ACT exp-only.
  E. Y = attn_out @ W_out + b_out in float32r (full-rate fp32 matmul).
"""

import sys

sys.path.insert(0, "/opt/trn_rl_repo")

import numpy as np

import concourse.bacc as bacc
import concourse.mybir as mybir
from concourse.bass_utils import run_bass_kernel_spmd
from concourse.masks import make_identity
from concourse.tile import TileContext

B = 8
S = 1024
D = 1024
H = 16
DK = D // H  # 64
P = 128
ST = S // P   # 8 s-tiles
DT = D // P   # 8 d-tiles
NTQK = 2 * D // P  # 16 n-tiles for the Q|K part
PAIRS = H // 2     # 8 head pairs
SC = S // 512      # 2 chunks of 512 (matmul free-dim limit)

f32 = mybir.dt.float32
f32r = mybir.dt.float32r
bf16 = mybir.dt.bfloat16
EXP = mybir.ActivationFunctionType.Exp
MULT = mybir.AluOpType.mult
ADD = mybir.AluOpType.add


def build_nc():
    nc = bacc.Bacc()
    X = nc.dram_tensor("X", [S, D], f32, kind="ExternalInput")
    W_in = nc.dram_tensor("W_in", [D, 3 * D], f32, kind="ExternalInput")
    b_in = nc.dram_tensor("b_in", [3 * D], f32, kind="ExternalInput")
    W_out = nc.dram_tensor("W_out", [D, D], f32, kind="ExternalInput")
    b_out = nc.dram_tensor("b_out", [D], f32, kind="ExternalInput")
    out = nc.dram_tensor("out", [S, D], f32, kind="ExternalOutput")

    w_in_kp = W_in.rearrange("(ko p) n -> p ko n", p=P)  # [128, 8, 3072]
    w_out_kp = W_out.rearrange("(ko p) n -> p ko n", p=P)  # [128, 8, 1024]

    with TileContext(nc) as tc:
        const = tc.alloc_tile_pool(name="const", bufs=1)
        # PSUM: wide pool ([128,1024] fp32 = 2 banks/slot, 2 slots) for
        # transposes/scores/C/E. The PV accumulators and the interleaved
        # B-projection accumulators are all [128,512] (1 bank) and share a
        # single-tag 4-slot ring: per sub-block the ring sees pv0, pv1,
        # bchunk3, bchunk7 in order, so each allocation's predecessor was
        # freed at least half a sub-block earlier (bias-adds get a full
        # sub-block) — no PSUM-slot wait can stall the in-order PE queue.
        # Total 4+4 = 8 banks.
        psum = tc.alloc_tile_pool(name="psum", bufs=2, space="PSUM")
        pvps = tc.alloc_tile_pool(name="pvps", bufs=4, space="PSUM")

        identity = const.tile([P, P], bf16)
        make_identity(nc, identity[:])
        bqk = const.tile([P, NTQK], f32)
        nc.sync.dma_start(bqk[:], b_in[0 : 2 * D].rearrange("(o p) -> p o", p=P))
        bv_bc = const.tile([P, D], f32)
        ones4 = const.tile([P, ST, H, 1], f32)
        nc.vector.memset(ones4[:], 1.0)

        # ---------------- resident tensors ----------------
        qkT_pool = tc.alloc_tile_pool(name="qkT", bufs=1)
        qkT = qkT_pool.tile([P, NTQK, S], bf16)  # 4 MB
        vaug_pool = tc.alloc_tile_pool(name="vaug", bufs=1)
        v_aug = vaug_pool.tile([P, ST, H, DK + 1], bf16)  # 2.1 MB
        nc.vector.tensor_copy(v_aug[:, :, :, DK : DK + 1], ones4[:])
        xT_pool = tc.alloc_tile_pool(name="xT", bufs=1)
        xT = xT_pool.tile([P, DT, S], bf16)  # 2 MB, lives through BD
        # attn^T and W_out in bf16: halves the normalize-multiply DVE cost
        # (16-bit output = 2 elem/cycle) and the SBUF footprint; phase E
        # runs as a bf16 matmul at the same PE rate
        attnT_pool = tc.alloc_tile_pool(name="attnT", bufs=1)
        attnT = attnT_pool.tile([P, DT, S], bf16)  # 2 MB
        wout_pool = tc.alloc_tile_pool(name="wout", bufs=1)
        wout = wout_pool.tile([P, DT, D], bf16)  # 2 MB; DMA+cast during BD

        # B-projection helpers: one n-tile (Q or K column block) of W_in.
        # stage_qk issues the DMA+cast (one pair of tiles ahead);
        # project_chunk emits one 8-matmul accumulation chunk, with the
        # DVE bias-add right after the last chunk so qkT(nt) is ready
        # well before the next pair's scores need it.
        def stage_qk(nt, wqk_pool):
            w_stage = wqk_pool.tile([P, DT, P], f32, tag="ws")
            nc.sync.dma_start(w_stage[:], w_in_kp[:, :, nt * P : (nt + 1) * P])
            w_tile = wqk_pool.tile([P, DT, P], bf16, tag="w")
            nc.vector.tensor_copy(w_tile[:], w_stage[:])
            return w_tile

        def project_chunk(nt, w_tile, ps_box, sc):
            ps = pvps.tile([P, 512], f32, tag="pv", name=f"psb{nt}")
            for dk in range(DT):
                nc.tensor.matmul(
                    ps[:],
                    w_tile[:, dk, :],
                    xT[:, dk, sc * 512 : (sc + 1) * 512],
                    start=(dk == 0),
                    stop=(dk == DT - 1),
                )
            # bias + cast on DVE per half (keeps ACT exp-only)
            nc.vector.tensor_scalar_add(
                qkT[:, nt, sc * 512 : (sc + 1) * 512], ps[:], bqk[:, nt : nt + 1]
            )

        def project_qk_tile(nt, wqk_pool):
            w_tile = stage_qk(nt, wqk_pool)
            for sc in range(SC):
                project_chunk(nt, w_tile, None, sc)

        # ---------------- phase A: X^T (PE transpose, cast to bf16) --------
        with tc.tile_pool(name="xstage", bufs=2) as xstage:
            bv_row = xstage.tile([1, D], f32, tag="brow")
            nc.sync.dma_start(bv_row[:], b_in[None, 2 * D : 3 * D])
            nc.gpsimd.partition_broadcast(bv_bc[:], bv_row[:])
            for si in range(ST):
                x_tile = xstage.tile([P, D], f32, tag="x")
                nc.sync.dma_start(x_tile[:], X[si * P : (si + 1) * P, :])
                xb = xstage.tile([P, D], bf16, tag="xb")
                nc.vector.tensor_copy(xb[:], x_tile[:])
                for dj in range(DT):
                    # transpose as a REGULAR bf16 matmul (x.T @ I): ~4x
                    # faster than fp32 transpose-mode and counts as PE
                    # activity for the HAM clock-gate warmup
                    tp = psum.tile([P, P], f32, tag="w", name="tp")
                    nc.tensor.matmul(
                        tp[:],
                        xb[:, dj * P : (dj + 1) * P],
                        identity[:],
                        start=True,
                        stop=True,
                    )
                    # split PSUM->SBUF copies across ACT and DVE
                    if dj % 2 == 0:
                        nc.scalar.copy(xT[:, dj, si * P : (si + 1) * P], tp[:])
                    else:
                        nc.vector.tensor_copy(
                            xT[:, dj, si * P : (si + 1) * P], tp[:]
                        )

        with (
            tc.tile_pool(name="wv", bufs=1) as wv,
            tc.tile_pool(name="wvstage", bufs=2) as wvstage,
            tc.tile_pool(name="wqk", bufs=2) as wqk_pool,
            tc.tile_pool(name="expp", bufs=5) as expp,
            tc.tile_pool(name="unp", bufs=2) as unp,
            tc.tile_pool(name="bcp", bufs=2) as bcp,
        ):
            # ---------------- B0: Q^T/K^T for pair 0 ----------------
            # emitted before C: B0's PE work runs while the V weights
            # stream in, and its wqk DMAs beat wv in the queue
            project_qk_tile(0, wqk_pool)
            project_qk_tile(PAIRS, wqk_pool)

            # V weights streamed per d-tile (full-D rows: 4 KB descriptor
            # rows; narrower slices halve effective DMA bandwidth).
            # Phase C runs in two dk-half contractions so it starts after
            # only half of W_v has landed (DMA is ~160 GB/s with 8 cores
            # sharing HBM); the dk0-3 partial is staged in bf16 v_part
            # (with the V bias folded in) and added in the second half.
            wv_tile = wv.tile([P, DT, D], bf16, tag="wv")
            v_part = wv.tile([P, ST, H, DK], bf16, tag="vpart")
            for half in range(2):
                for dk in range(4 * half, 4 * half + 4):
                    wv_stage = wvstage.tile([P, D], f32, tag="wvs")
                    nc.sync.dma_start(
                        wv_stage[:], w_in_kp[:, dk, 2 * D : 3 * D]
                    )
                    nc.vector.tensor_copy(wv_tile[:, dk, :], wv_stage[:])
                for st in range(ST):
                    ps = psum.tile([P, D], f32, tag="w", name="psc")
                    for dk in range(4 * half, 4 * half + 4):
                        nc.tensor.matmul(
                            ps[:, 0:512],
                            xT[:, dk, st * P : (st + 1) * P],
                            wv_tile[:, dk, 0:512],
                            start=(dk == 4 * half),
                            stop=(dk == 4 * half + 3),
                        )
                        nc.tensor.matmul(
                            ps[:, 512:1024],
                            xT[:, dk, st * P : (st + 1) * P],
                            wv_tile[:, dk, 512:1024],
                            start=(dk == 4 * half),
                            stop=(dk == 4 * half + 3),
                        )
                    if half == 0:
                        nc.vector.tensor_tensor(
                            v_part[:, st],
                            ps[:].rearrange("p (h d) -> p h d", d=DK),
                            bv_bc[:].rearrange("p (h d) -> p h d", d=DK),
                            ADD,
                        )
                    else:
                        nc.vector.tensor_tensor(
                            v_aug[:, st, :, 0:DK],
                            ps[:].rearrange("p (h d) -> p h d", d=DK),
                            v_part[:, st],
                            ADD,
                        )

            # ---------------- BD: attention ∥ next-pair projection ---------
            def emit_scores(pr, sc, sk):
                sps = psum.tile([P, S], f32, tag="w", name="sps")
                for hh in range(2):
                    base = hh * DK
                    nc.tensor.matmul(
                        sps[:, hh * 512 : (hh + 1) * 512],
                        qkT[
                            base : base + DK,
                            PAIRS + pr,
                            sk * P : (sk + 1) * P,
                        ],
                        qkT[base : base + DK, pr, sc * 512 : (sc + 1) * 512],
                        start=True,
                        stop=True,
                    )
                ex = expp.tile([P, S], bf16, tag="ex")
                nc.scalar.activation(ex[:], sps[:], EXP, scale=1.0 / np.sqrt(DK))
                return ex

            SBS = [(pr, sc) for pr in range(PAIRS) for sc in range(SC)]
            carry_ex0 = None
            w_next = None
            for idx, (pr, sc) in enumerate(SBS):
                # stage next pair's Q/K weights (DMA+cast run a pair ahead)
                if sc == 0 and pr + 1 < PAIRS:
                    w_next = [
                        stage_qk(pr + 1, wqk_pool),
                        stage_qk(PAIRS + pr + 1, wqk_pool),
                    ]
                nt_next = (pr + 1) if sc == 0 else (PAIRS + pr + 1)
                pv = [
                    pvps.tile([P, 512], f32, tag="pv", name=f"pv{i}")
                    for i in range(2)
                ]
                exps = {}
                # software pipeline: paired scores(sk) on PE, one wide
                # exp(sk) on ACT (hh halves share the tile), pv(sk-1);
                # next pair's projection chunks at sk 4/6 fill the PE
                # slack of this ACT-bound loop. The first scores of the
                # NEXT sub-block are emitted before PV(7) so ACT never
                # waits on the exp->PV(7)->scores serial tail.
                if carry_ex0 is not None:
                    exps[0] = carry_ex0
                    carry_ex0 = None
                else:
                    exps[0] = emit_scores(pr, sc, 0)
                for sk in range(1, ST + 1):
                    if sk < ST:
                        exps[sk] = emit_scores(pr, sc, sk)
                    if sk == ST - 1 and idx + 1 < len(SBS):
                        # pre-emit next sub-block's scores(0)+exp at high
                        # priority: the list scheduler must place it BEFORE
                        # this block's last PVs (which stall on exp) or ACT
                        # idles ~1.4us at every sub-block boundary
                        npr, nsc = SBS[idx + 1]
                        with tc.high_priority(offset=80):
                            carry_ex0 = emit_scores(npr, nsc, 0)
                    ex = exps.pop(sk - 1)
                    for hh in range(2):
                        h = 2 * pr + hh
                        nc.tensor.matmul(
                            pv[hh][0 : DK + 1, :],
                            v_aug[:, sk - 1, h, :],
                            ex[:, hh * 512 : (hh + 1) * 512],
                            start=(sk - 1 == 0),
                            stop=(sk - 1 == ST - 1),
                        )
                    if pr + 1 < PAIRS and sk in (4, 6):
                        project_chunk(
                            nt_next,
                            w_next[sc],
                            None,
                            0 if sk == 4 else 1,
                        )
                    # normalize. un copies emitted first: they are the only
                    # PSUM readers, so the pv slots release ~0.7/1.4us
                    # after stop regardless of the rest of the chain. Denom
                    # row to partition 0 (plain DVE copy does the p64->p0
                    # move; both the custom-ucode fast reciprocal and the
                    # GpSimd broadcast need partition-0 inputs). The final
                    # multiply runs on GpSimd, keeping DVE under ~60%.
                    uns, dns, rrs, bcs = [], [], [], []
                    for hh in range(2):
                        un = unp.tile([DK + 1, 512], f32, tag="un")
                        nc.vector.tensor_copy(un[:], pv[hh][0 : DK + 1, :])
                        uns.append(un)
                    for hh in range(2):
                        dn = bcp.tile([1, 512], f32, tag=f"dnrr{hh}")
                        nc.vector.tensor_copy(dn[:], uns[hh][DK : DK + 1, :])
                        dns.append(dn)
                    for hh in range(2):
                        rrow = bcp.tile([1, 512], f32, tag=f"dnrr{hh}")
                        nc.vector.reciprocal_approx_fast(rrow[:], dns[hh][:])
                        rrs.append(rrow)
                    for hh in range(2):
                        # full-tile broadcast (sliced outputs break on HW)
                        bc = bcp.tile([P, 512], f32, tag="bc")
                        nc.gpsimd.partition_broadcast(bc[:], rrs[hh][:])
                        bcs.append(bc)
                    for hh in range(2):
                        base = hh * DK
                        nc.vector.tensor_tensor(
                            attnT[
                                base : base + DK, pr, sc * 512 : (sc + 1) * 512
                            ],
                            uns[hh][0:DK, :],
                            bcs[hh][0:DK, :],
                            MULT,
                        )
                    # W_out prefetch + bf16 cast mid-loop (one d-tile per
                    # sub-block through the wv stage pool): after the front
                    # DMA burst drains, well before phase E needs it
                    if pr >= 2 and pr <= 5:
                        dk = 2 * (pr - 2) + sc
                        wo_stage = wvstage.tile([P, D], f32, tag="wvs")
                        nc.sync.dma_start(wo_stage[:], w_out_kp[:, dk, :])
                        nc.vector.tensor_copy(wout[:, dk, :], wo_stage[:])

        # ---------------- phase E: output projection (f32r) ----------------
        with tc.tile_pool(name="ypool", bufs=3) as ypool:
            # b_out broadcast staged here (GpSimd is idle during E);
            # keeps 4 KB/partition free during the BD loop
            bout_bc = ypool.tile([P, D], f32, tag="bout")
            bout_row = ypool.tile([1, D], f32, tag="boutrow")
            nc.sync.dma_start(bout_row[:], b_out[None, :])
            nc.gpsimd.partition_broadcast(bout_bc[:], bout_row[:])
            for st in range(ST):
                ps = psum.tile([P, D], f32, tag="w", name="pse")
                for ncx in range(SC):
                    for dk in range(DT):
                        nc.tensor.matmul(
                            ps[:, ncx * 512 : (ncx + 1) * 512],
                            attnT[:, dk, st * P : (st + 1) * P],
                            wout[:, dk, ncx * 512 : (ncx + 1) * 512],
                            start=(dk == 0),
                            stop=(dk == DT - 1),
                        )
                y = ypool.tile([P, D], f32, tag="y")
                nc.vector.tensor_tensor(y[:], ps[:], bout_bc[:], ADD)
                nc.sync.dma_start(out[st * P : (st + 1) * P, :], y[:])

        for pool in (
            wout_pool,
            attnT_pool,
            xT_pool,
            vaug_pool,
            qkT_pool,
            pvps,
            psum,
            const,
        ):
            pool.release()

    nc.finalize()
    return nc


_NC_CACHE = {}


def get_nc():
    if "nc" not in _NC_CACHE:
        _NC_CACHE["nc"] = build_nc()
    return _NC_CACHE["nc"]


def kernel(X, W_in, b_in, W_out, b_out):
    X = np.ascontiguousarray(np.asarray(X, dtype=np.float32))
    W_in = np.ascontiguousarray(np.asarray(W_in, dtype=np.float32))
    b_in = np.ascontiguousarray(np.asarray(b_in, dtype=np.float32))
    W_out = np.ascontiguousarray(np.asarray(W_out, dtype=np.float32))
    b_out = np.ascontiguousarray(np.asarray(b_out, dtype=np.float32))

    nc = get_nc()
    in_maps = [
        {"X": X[i], "W_in": W_in, "b_in": b_in, "W_out": W_out, "b_out": b_out}
        for i in range(B)
    ]
    res = run_bass_kernel_spmd(nc, in_maps, core_ids=list(range(B)))
    return np.stack([res.results[i]["out"] for i in range(B)], axis=0)


# revision 47
# speedup vs baseline: 1.0180x; 1.0180x over previous
"""Multi-head attention (B=8, S=1024, D=1024, H=16) on 8 TRN2 NeuronCores.

Sharding: pure data parallel — batch element b on core b. Weights are
broadcast to every core. No collectives.

Per-core pipeline (X: [S, D] for one batch element):
  A. X^T via PE transposes (bf16 matmul vs identity), PSUM->SBUF copies
     split between ACT and DVE.
  C. V = X @ W_v (bf16, natural layout) stored as V_aug[sk, head, 65]
     with a ones column (col 64) that accumulates the softmax denominator
     during PV.
  B0. Q^T/K^T projection for head pair 0 only.
  BD. For each head pair p: attention block for p interleaved (in PE
     program order) with the Q^T/K^T projection of pair p+1. The
     attention block is ACT-bound (8 wide exps per sc chunk ~= 8.6us vs
     5us of PE scores+PV), so pair p+1's projection matmuls fill the PE
     slack instead of forming a separate PE-bound phase.
       scores^T (paired across the 2 heads: 64-row PE groups run
       concurrently), one wide exp per sk on ACT (scale=1/8,
       max-subtraction skipped: scores ~N(0,1)), PV accumulates
       unnormalized out^T (rows 0-63) + denominator (row 64).
     Normalize per (pair, sc, head): DVE copy PSUM->SBUF (fast PSUM bank
     release), reciprocal_approx_fast on the denom row, GpSimd
     partition-broadcast (idle engine), DVE multiply -> attn^T (f32r).
     All bias-adds ride on DVE (tensor_scalar/tensor_tensor), keeping
     ACT exp-only.
  E. Y = attn_out @ W_out + b_out in float32r (full-rate fp32 matmul).
"""

import sys

sys.path.insert(0, "/opt/trn_rl_repo")

import numpy as np

import concourse.bacc as bacc
import concourse.mybir as mybir
from concourse.bass_utils import run_bass_kernel_spmd
from concourse.masks import make_identity
from concourse.tile import TileContext

B = 8
S = 1024
D = 1024
H = 16
DK = D // H  # 64
P = 128
ST = S // P   # 8 s-tiles
DT = D // P   # 8 d-tiles
NTQK = 2 * D // P  # 16 n-tiles for the Q|K part
PAIRS = H // 2     # 8 head pairs
SC = S // 512      # 2 chunks of 512 (matmul free-dim limit)

f32 = mybir.dt.float32
f32r = mybir.dt.float32r
bf16 = mybir.dt.bfloat16
EXP = mybir.ActivationFunctionType.Exp
MULT = mybir.AluOpType.mult
ADD = mybir.AluOpType.add


def build_nc():
    nc = bacc.Bacc()
    X = nc.dram_tensor("X", [S, D], f32, kind="ExternalInput")
    W_in = nc.dram_tensor("W_in", [D, 3 * D], f32, kind="ExternalInput")
    b_in = nc.dram_tensor("b_in", [3 * D], f32, kind="ExternalInput")
    W_out = nc.dram_tensor("W_out", [D, D], f32, kind="ExternalInput")
    b_out = nc.dram_tensor("b_out", [D], f32, kind="ExternalInput")
    out = nc.dram_tensor("out", [S, D], f32, kind="ExternalOutput")

    w_in_kp = W_in.rearrange("(ko p) n -> p ko n", p=P)  # [128, 8, 3072]
    w_out_kp = W_out.rearrange("(ko p) n -> p ko n", p=P)  # [128, 8, 1024]

    with TileContext(nc) as tc:
        const = tc.alloc_tile_pool(name="const", bufs=1)
        # PSUM: wide pool ([128,1024] fp32 = 2 banks/slot, 2 slots) for
        # transposes/scores/C/E. The PV accumulators and the interleaved
        # B-projection accumulators are all [128,512] (1 bank) and share a
        # single-tag 4-slot ring: per sub-block the ring sees pv0, pv1,
        # bchunk3, bchunk7 in order, so each allocation's predecessor was
        # freed at least half a sub-block earlier (bias-adds get a full
        # sub-block) — no PSUM-slot wait can stall the in-order PE queue.
        # Total 4+4 = 8 banks.
        psum = tc.alloc_tile_pool(name="psum", bufs=2, space="PSUM")
        pvps = tc.alloc_tile_pool(name="pvps", bufs=4, space="PSUM")

        identity = const.tile([P, P], bf16)
        make_identity(nc, identity[:])
        bqk = const.tile([P, NTQK], f32)
        nc.sync.dma_start(bqk[:], b_in[0 : 2 * D].rearrange("(o p) -> p o", p=P))
        bv_bc = const.tile([P, D], f32)
        ones4 = const.tile([P, ST, H, 1], f32)
        nc.vector.memset(ones4[:], 1.0)

        # ---------------- resident tensors ----------------
        qkT_pool = tc.alloc_tile_pool(name="qkT", bufs=1)
        qkT = qkT_pool.tile([P, NTQK, S], bf16)  # 4 MB
        vaug_pool = tc.alloc_tile_pool(name="vaug", bufs=1)
        v_aug = vaug_pool.tile([P, ST, H, DK + 1], bf16)  # 2.1 MB
        nc.vector.tensor_copy(v_aug[:, :, :, DK : DK + 1], ones4[:])
        xT_pool = tc.alloc_tile_pool(name="xT", bufs=1)
        xT = xT_pool.tile([P, DT, S], bf16)  # 2 MB, lives through BD
        # attn^T and W_out in bf16: halves the normalize-multiply DVE cost
        # (16-bit output = 2 elem/cycle) and the SBUF footprint; phase E
        # runs as a bf16 matmul at the same PE rate
        attnT_pool = tc.alloc_tile_pool(name="attnT", bufs=1)
        attnT = attnT_pool.tile([P, DT, S], bf16)  # 2 MB
        wout_pool = tc.alloc_tile_pool(name="wout", bufs=1)
        wout = wout_pool.tile([P, DT, D], bf16)  # 2 MB; DMA+cast during BD

        # B-projection helpers: one n-tile (Q or K column block) of W_in.
        # stage_qk issues the DMA+cast (one pair of tiles ahead);
        # project_chunk emits one 8-matmul accumulation chunk, with the
        # DVE bias-add right after the last chunk so qkT(nt) is ready
        # well before the next pair's scores need it.
        def stage_qk(nt, wqk_pool):
            w_stage = wqk_pool.tile([P, DT, P], f32, tag="ws")
            nc.sync.dma_start(w_stage[:], w_in_kp[:, :, nt * P : (nt + 1) * P])
            w_tile = wqk_pool.tile([P, DT, P], bf16, tag="w")
            nc.vector.tensor_copy(w_tile[:], w_stage[:])
            return w_tile

        def project_chunk(nt, w_tile, ps_box, sc):
            ps = pvps.tile([P, 512], f32, tag="pv", name=f"psb{nt}")
            for dk in range(DT):
                nc.tensor.matmul(
                    ps[:],
                    w_tile[:, dk, :],
                    xT[:, dk, sc * 512 : (sc + 1) * 512],
                    start=(dk == 0),
                    stop=(dk == DT - 1),
                )
            # bias + cast on DVE per half (keeps ACT exp-only)
            nc.vector.tensor_scalar_add(
                qkT[:, nt, sc * 512 : (sc + 1) * 512], ps[:], bqk[:, nt : nt + 1]
            )

        def project_qk_tile(nt, wqk_pool):
            w_tile = stage_qk(nt, wqk_pool)
            for sc in range(SC):
                project_chunk(nt, w_tile, None, sc)

        # ---------------- phase A: X^T (PE transpose, cast to bf16) --------
        with tc.tile_pool(name="xstage", bufs=2) as xstage:
            bv_row = xstage.tile([1, D], f32, tag="brow")
            nc.sync.dma_start(bv_row[:], b_in[None, 2 * D : 3 * D])
            nc.gpsimd.partition_broadcast(bv_bc[:], bv_row[:])
            for si in range(ST):
                x_tile = xstage.tile([P, D], f32, tag="x")
                nc.sync.dma_start(x_tile[:], X[si * P : (si + 1) * P, :])
                xb = xstage.tile([P, D], bf16, tag="xb")
                nc.vector.tensor_copy(xb[:], x_tile[:])
                for dj in range(DT):
                    # transpose as a REGULAR bf16 matmul (x.T @ I): ~4x
                    # faster than fp32 transpose-mode and counts as PE
                    # activity for the HAM clock-gate warmup
                    tp = psum.tile([P, P], f32, tag="w", name="tp")
                    nc.tensor.matmul(
                        tp[:],
                        xb[:, dj * P : (dj + 1) * P],
                        identity[:],
                        start=True,
                        stop=True,
                    )
                    # split PSUM->SBUF copies across ACT and DVE
                    if dj % 2 == 0:
                        nc.scalar.copy(xT[:, dj, si * P : (si + 1) * P], tp[:])
                    else:
                        nc.vector.tensor_copy(
                            xT[:, dj, si * P : (si + 1) * P], tp[:]
                        )

        with (
            tc.tile_pool(name="wv", bufs=1) as wv,
            tc.tile_pool(name="wvstage", bufs=2) as wvstage,
            tc.tile_pool(name="wqk", bufs=2) as wqk_pool,
            tc.tile_pool(name="expp", bufs=5) as expp,
            tc.tile_pool(name="unp", bufs=2) as unp,
            tc.tile_pool(name="bcp", bufs=2) as bcp,
        ):
            # ---------------- B0: Q^T/K^T for pair 0 ----------------
            # emitted before C: B0's PE work runs while the V weights
            # stream in, and its wqk DMAs beat wv in the queue
            project_qk_tile(0, wqk_pool)
            project_qk_tile(PAIRS, wqk_pool)

            # V weights streamed per d-tile (full-D rows: 4 KB descriptor
            # rows; narrower slices halve effective DMA bandwidth).
            # Phase C runs in two dk-half contractions so it starts after
            # only half of W_v has landed (DMA is ~160 GB/s with 8 cores
            # sharing HBM); the dk0-3 partial is staged in bf16 v_part
            # (with the V bias folded in) and added in the second half.
            wv_tile = wv.tile([P, DT, D], bf16, tag="wv")
            v_part = wv.tile([P, ST, H, DK], bf16, tag="vpart")
            for half in range(2):
                for dk in range(4 * half, 4 * half + 4):
                    wv_stage = wvstage.tile([P, D], f32, tag="wvs")
                    nc.sync.dma_start(
                        wv_stage[:], w_in_kp[:, dk, 2 * D : 3 * D]
                    )
                    nc.vector.tensor_copy(wv_tile[:, dk, :], wv_stage[:])
                for st in range(ST):
                    ps = psum.tile([P, D], f32, tag="w", name="psc")
                    for dk in range(4 * half, 4 * half + 4):
                        nc.tensor.matmul(
                            ps[:, 0:512],
                            xT[:, dk, st * P : (st + 1) * P],
                            wv_tile[:, dk, 0:512],
                            start=(dk == 4 * half),
                            stop=(dk == 4 * half + 3),
                        )
                        nc.tensor.matmul(
                            ps[:, 512:1024],
                            xT[:, dk, st * P : (st + 1) * P],
                            wv_tile[:, dk, 512:1024],
                            start=(dk == 4 * half),
                            stop=(dk == 4 * half + 3),
                        )
                    if half == 0:
                        nc.vector.tensor_tensor(
                            v_part[:, st],
                            ps[:].rearrange("p (h d) -> p h d", d=DK),
                            bv_bc[:].rearrange("p (h d) -> p h d", d=DK),
                            ADD,
                        )
                    else:
                        nc.vector.tensor_tensor(
                            v_aug[:, st, :, 0:DK],
                            ps[:].rearrange("p (h d) -> p h d", d=DK),
                            v_part[:, st],
                            ADD,
                        )

            # ---------------- BD: attention ∥ next-pair projection ---------
            def emit_scores(pr, sc, sk):
                sps = psum.tile([P, S], f32, tag="w", name="sps")
                for hh in range(2):
                    base = hh * DK
                    nc.tensor.matmul(
                        sps[:, hh * 512 : (hh + 1) * 512],
                        qkT[
                            base : base + DK,
                            PAIRS + pr,
                            sk * P : (sk + 1) * P,
                        ],
                        qkT[base : base + DK, pr, sc * 512 : (sc + 1) * 512],
                        start=True,
                        stop=True,
                    )
                ex = expp.tile([P, S], bf16, tag="ex")
                nc.scalar.activation(ex[:], sps[:], EXP, scale=1.0 / np.sqrt(DK))
                return ex

            SBS = [(pr, sc) for pr in range(PAIRS) for sc in range(SC)]
            carry_ex0 = None
            w_next = None
            for idx, (pr, sc) in enumerate(SBS):
                # stage next pair's Q/K weights (DMA+cast run a pair ahead)
                if sc == 0 and pr + 1 < PAIRS:
                    w_next = [
                        stage_qk(pr + 1, wqk_pool),
                        stage_qk(PAIRS + pr + 1, wqk_pool),
                    ]
                nt_next = (pr + 1) if sc == 0 else (PAIRS + pr + 1)
                pv = [
                    pvps.tile([P, 512], f32, tag="pv", name=f"pv{i}")
                    for i in range(2)
                ]
                exps = {}
                # software pipeline: paired scores(sk) on PE, one wide
                # exp(sk) on ACT (hh halves share the tile), pv(sk-1);
                # next pair's projection chunks at sk 4/6 fill the PE
                # slack of this ACT-bound loop. The first scores of the
                # NEXT sub-block are emitted before PV(7) so ACT never
                # waits on the exp->PV(7)->scores serial tail.
                if carry_ex0 is not None:
                    exps[0] = carry_ex0
                    carry_ex0 = None
                else:
                    exps[0] = emit_scores(pr, sc, 0)
                for sk in range(1, ST + 1):
                    if sk < ST:
                        exps[sk] = emit_scores(pr, sc, sk)
                    if sk == ST - 1 and idx + 1 < len(SBS):
                        # pre-emit next sub-block's scores(0)+exp at high
                        # priority: the list scheduler must place it BEFORE
                        # this block's last PVs (which stall on exp) or ACT
                        # idles ~1.4us at every sub-block boundary
                        npr, nsc = SBS[idx + 1]
                        with tc.high_priority(offset=12):
                            carry_ex0 = emit_scores(npr, nsc, 0)
                    ex = exps.pop(sk - 1)
                    for hh in range(2):
                        h = 2 * pr + hh
                        nc.tensor.matmul(
                            pv[hh][0 : DK + 1, :],
                            v_aug[:, sk - 1, h, :],
                            ex[:, hh * 512 : (hh + 1) * 512],
                            start=(sk - 1 == 0),
                            stop=(sk - 1 == ST - 1),
                        )
                    if pr + 1 < PAIRS and sk in (4, 6):
                        project_chunk(
                            nt_next,
                            w_next[sc],
                            None,
                            0 if sk == 4 else 1,
                        )
                    # normalize. un copies emitted first: they are the only
                    # PSUM readers, so the pv slots release ~0.7/1.4us
                    # after stop regardless of the rest of the chain. Denom
                    # row to partition 0 (plain DVE copy does the p64->p0
                    # move; both the custom-ucode fast reciprocal and the
                    # GpSimd broadcast need partition-0 inputs). The final
                    # multiply runs on GpSimd, keeping DVE under ~60%.
                    uns, dns, rrs, bcs = [], [], [], []
                    for hh in range(2):
                        un = unp.tile([DK + 1, 512], f32, tag="un")
                        nc.vector.tensor_copy(un[:], pv[hh][0 : DK + 1, :])
                        uns.append(un)
                    for hh in range(2):
                        dn = bcp.tile([1, 512], f32, tag=f"dnrr{hh}")
                        nc.vector.tensor_copy(dn[:], uns[hh][DK : DK + 1, :])
                        dns.append(dn)
                    for hh in range(2):
                        rrow = bcp.tile([1, 512], f32, tag=f"dnrr{hh}")
                        nc.vector.reciprocal_approx_fast(rrow[:], dns[hh][:])
                        rrs.append(rrow)
                    for hh in range(2):
                        # full-tile broadcast (sliced outputs break on HW)
                        bc = bcp.tile([P, 512], f32, tag="bc")
                        nc.gpsimd.partition_broadcast(bc[:], rrs[hh][:])
                        bcs.append(bc)
                    for hh in range(2):
                        base = hh * DK
                        nc.vector.tensor_tensor(
                            attnT[
                                base : base + DK, pr, sc * 512 : (sc + 1) * 512
                            ],
                            uns[hh][0:DK, :],
                            bcs[hh][0:DK, :],
                            MULT,
                        )
                    # W_out prefetch + bf16 cast mid-loop (one d-tile per
                    # sub-block through the wv stage pool): after the front
                    # DMA burst drains, well before phase E needs it
                    if pr >= 2 and pr <= 5:
                        dk = 2 * (pr - 2) + sc
                        wo_stage = wvstage.tile([P, D], f32, tag="wvs")
                        nc.sync.dma_start(wo_stage[:], w_out_kp[:, dk, :])
                        nc.vector.tensor_copy(wout[:, dk, :], wo_stage[:])

        # ---------------- phase E: output projection (f32r) ----------------
        with tc.tile_pool(name="ypool", bufs=3) as ypool:
            # b_out broadcast staged here (GpSimd is idle during E);
            # keeps 4 KB/partition free during the BD loop
            bout_bc = ypool.tile([P, D], f32, tag="bout")
            bout_row = ypool.tile([1, D], f32, tag="boutrow")
            nc.sync.dma_start(bout_row[:], b_out[None, :])
            nc.gpsimd.partition_broadcast(bout_bc[:], bout_row[:])
            for st in range(ST):
                ps = psum.tile([P, D], f32, tag="w", name="pse")
                for ncx in range(SC):
                    for dk in range(DT):
                        nc.tensor.matmul(
                            ps[:, ncx * 512 : (ncx + 1) * 512],
                            attnT[:, dk, st * P : (st + 1) * P],
                            wout[:, dk, ncx * 512 : (ncx + 1) * 512],
                            start=(dk == 0),
                            stop=(dk == DT - 1),
                        )
                y = ypool.tile([P, D], f32, tag="y")
                nc.vector.tensor_tensor(y[:], ps[:], bout_bc[:], ADD)
                nc.sync.dma_start(out[st * P : (st + 1) * P, :], y[:])

        for pool in (
            wout_pool,
            attnT_pool,
            xT_pool,
            vaug_pool,
            qkT_pool,
            pvps,
            psum,
            const,
        ):
            pool.release()

    nc.finalize()
    return nc


_NC_CACHE = {}


def get_nc():
    if "nc" not in _NC_CACHE:
        _NC_CACHE["nc"] = build_nc()
    return _NC_CACHE["nc"]


def kernel(X, W_in, b_in, W_out, b_out):
    X = np.ascontiguousarray(np.asarray(X, dtype=np.float32))
    W_in = np.ascontiguousarray(np.asarray(W_in, dtype=np.float32))
    b_in = np.ascontiguousarray(np.asarray(b_in, dtype=np.float32))
    W_out = np.ascontiguousarray(np.asarray(W_out, dtype=np.float32))
    b_out = np.ascontiguousarray(np.asarray(b_out, dtype=np.float32))

    nc = get_nc()
    in_maps = [
        {"X": X[i], "W_in": W_in, "b_in": b_in, "W_out": W_out, "b_out": b_out}
        for i in range(B)
    ]
    res = run_bass_kernel_spmd(nc, in_maps, core_ids=list(range(B)))
    return np.stack([res.results[i]["out"] for i in range(B)], axis=0)


# revision 48
# speedup vs baseline: 1.0433x; 1.0249x over previous
"""Multi-head attention (B=8, S=1024, D=1024, H=16) on 8 TRN2 NeuronCores.

Sharding: pure data parallel — batch element b on core b. Weights are
broadcast to every core. No collectives.

Per-core pipeline (X: [S, D] for one batch element):
  A. X^T via PE transposes (bf16 matmul vs identity), PSUM->SBUF copies
     split between ACT and DVE.
  C. V = X @ W_v (bf16, natural layout) stored as V_aug[sk, head, 65]
     with a ones column (col 64) that accumulates the softmax denominator
     during PV.
  B0. Q^T/K^T projection for head pair 0 only.
  BD. For each head pair p: attention block for p interleaved (in PE
     program order) with the Q^T/K^T projection of pair p+1. The
     attention block is ACT-bound (8 wide exps per sc chunk ~= 8.6us vs
     5us of PE scores+PV), so pair p+1's projection matmuls fill the PE
     slack instead of forming a separate PE-bound phase.
       scores^T (paired across the 2 heads: 64-row PE groups run
       concurrently), one wide exp per sk on ACT (scale=1/8,
       max-subtraction skipped: scores ~N(0,1)), PV accumulates
       unnormalized out^T (rows 0-63) + denominator (row 64).
     Normalize per (pair, sc, head): DVE copy PSUM->SBUF (fast PSUM bank
     release), reciprocal_approx_fast on the denom row, GpSimd
     partition-broadcast (idle engine), DVE multiply -> attn^T (f32r).
     All bias-adds ride on DVE (tensor_scalar/tensor_tensor), keeping
     ACT exp-only.
  E. Y = attn_out @ W_out + b_out in float32r (full-rate fp32 matmul).
"""

import sys

sys.path.insert(0, "/opt/trn_rl_repo")

import numpy as np

import concourse.bacc as bacc
import concourse.mybir as mybir
from concourse.bass_utils import run_bass_kernel_spmd
from concourse.masks import make_identity
from concourse.tile import TileContext

B = 8
S = 1024
D = 1024
H = 16
DK = D // H  # 64
P = 128
ST = S // P   # 8 s-tiles
DT = D // P   # 8 d-tiles
NTQK = 2 * D // P  # 16 n-tiles for the Q|K part
PAIRS = H // 2     # 8 head pairs
SC = S // 512      # 2 chunks of 512 (matmul free-dim limit)

f32 = mybir.dt.float32
f32r = mybir.dt.float32r
bf16 = mybir.dt.bfloat16
EXP = mybir.ActivationFunctionType.Exp
MULT = mybir.AluOpType.mult
ADD = mybir.AluOpType.add


def build_nc():
    nc = bacc.Bacc()
    X = nc.dram_tensor("X", [S, D], f32, kind="ExternalInput")
    W_in = nc.dram_tensor("W_in", [D, 3 * D], f32, kind="ExternalInput")
    b_in = nc.dram_tensor("b_in", [3 * D], f32, kind="ExternalInput")
    W_out = nc.dram_tensor("W_out", [D, D], f32, kind="ExternalInput")
    b_out = nc.dram_tensor("b_out", [D], f32, kind="ExternalInput")
    out = nc.dram_tensor("out", [S, D], f32, kind="ExternalOutput")

    w_in_kp = W_in.rearrange("(ko p) n -> p ko n", p=P)  # [128, 8, 3072]
    w_out_kp = W_out.rearrange("(ko p) n -> p ko n", p=P)  # [128, 8, 1024]

    with TileContext(nc) as tc:
        const = tc.alloc_tile_pool(name="const", bufs=1)
        # PSUM: wide pool ([128,1024] fp32 = 2 banks/slot, 2 slots) for
        # transposes/scores/C/E. The PV accumulators and the interleaved
        # B-projection accumulators are all [128,512] (1 bank) and share a
        # single-tag 4-slot ring: per sub-block the ring sees pv0, pv1,
        # bchunk3, bchunk7 in order, so each allocation's predecessor was
        # freed at least half a sub-block earlier (bias-adds get a full
        # sub-block) — no PSUM-slot wait can stall the in-order PE queue.
        # Total 4+4 = 8 banks.
        psum = tc.alloc_tile_pool(name="psum", bufs=2, space="PSUM")
        pvps = tc.alloc_tile_pool(name="pvps", bufs=4, space="PSUM")

        identity = const.tile([P, P], bf16)
        make_identity(nc, identity[:])
        bqk = const.tile([P, NTQK], f32)
        nc.sync.dma_start(bqk[:], b_in[0 : 2 * D].rearrange("(o p) -> p o", p=P))
        bv_bc = const.tile([P, D], f32)
        ones4 = const.tile([P, ST, H, 1], f32)
        nc.vector.memset(ones4[:], 1.0)

        # ---------------- resident tensors ----------------
        qkT_pool = tc.alloc_tile_pool(name="qkT", bufs=1)
        qkT = qkT_pool.tile([P, NTQK, S], bf16)  # 4 MB
        vaug_pool = tc.alloc_tile_pool(name="vaug", bufs=1)
        v_aug = vaug_pool.tile([P, ST, H, DK + 1], bf16)  # 2.1 MB
        nc.vector.tensor_copy(v_aug[:, :, :, DK : DK + 1], ones4[:])
        xT_pool = tc.alloc_tile_pool(name="xT", bufs=1)
        xT = xT_pool.tile([P, DT, S], bf16)  # 2 MB, lives through BD
        # attn^T and W_out in bf16: halves the normalize-multiply DVE cost
        # (16-bit output = 2 elem/cycle) and the SBUF footprint; phase E
        # runs as a bf16 matmul at the same PE rate
        attnT_pool = tc.alloc_tile_pool(name="attnT", bufs=1)
        attnT = attnT_pool.tile([P, DT, S], bf16)  # 2 MB
        wout_pool = tc.alloc_tile_pool(name="wout", bufs=1)
        wout = wout_pool.tile([P, DT, D], bf16)  # 2 MB; DMA+cast during BD

        # B-projection helpers: one n-tile (Q or K column block) of W_in.
        # stage_qk issues the DMA+cast (one pair of tiles ahead);
        # project_chunk emits one 8-matmul accumulation chunk, with the
        # DVE bias-add right after the last chunk so qkT(nt) is ready
        # well before the next pair's scores need it.
        def stage_qk(nt, wqk_pool):
            w_stage = wqk_pool.tile([P, DT, P], f32, tag="ws")
            nc.sync.dma_start(w_stage[:], w_in_kp[:, :, nt * P : (nt + 1) * P])
            w_tile = wqk_pool.tile([P, DT, P], bf16, tag="w")
            nc.vector.tensor_copy(w_tile[:], w_stage[:])
            return w_tile

        def project_chunk(nt, w_tile, ps_box, sc):
            ps = pvps.tile([P, 512], f32, tag="pv", name=f"psb{nt}")
            for dk in range(DT):
                nc.tensor.matmul(
                    ps[:],
                    w_tile[:, dk, :],
                    xT[:, dk, sc * 512 : (sc + 1) * 512],
                    start=(dk == 0),
                    stop=(dk == DT - 1),
                )
            # bias + cast on DVE per half (keeps ACT exp-only)
            nc.vector.tensor_scalar_add(
                qkT[:, nt, sc * 512 : (sc + 1) * 512], ps[:], bqk[:, nt : nt + 1]
            )

        def project_qk_tile(nt, wqk_pool):
            w_tile = stage_qk(nt, wqk_pool)
            for sc in range(SC):
                project_chunk(nt, w_tile, None, sc)

        # ---------------- phase A: X^T (PE transpose, cast to bf16) --------
        with tc.tile_pool(name="xstage", bufs=2) as xstage:
            bv_row = xstage.tile([1, D], f32, tag="brow")
            nc.sync.dma_start(bv_row[:], b_in[None, 2 * D : 3 * D])
            nc.gpsimd.partition_broadcast(bv_bc[:], bv_row[:])
            for si in range(ST):
                x_tile = xstage.tile([P, D], f32, tag="x")
                nc.sync.dma_start(x_tile[:], X[si * P : (si + 1) * P, :])
                xb = xstage.tile([P, D], bf16, tag="xb")
                nc.vector.tensor_copy(xb[:], x_tile[:])
                for dj in range(DT):
                    # transpose as a REGULAR bf16 matmul (x.T @ I): ~4x
                    # faster than fp32 transpose-mode and counts as PE
                    # activity for the HAM clock-gate warmup
                    tp = psum.tile([P, P], f32, tag="w", name="tp")
                    nc.tensor.matmul(
                        tp[:],
                        xb[:, dj * P : (dj + 1) * P],
                        identity[:],
                        start=True,
                        stop=True,
                    )
                    # split PSUM->SBUF copies across ACT and DVE
                    if dj % 2 == 0:
                        nc.scalar.copy(xT[:, dj, si * P : (si + 1) * P], tp[:])
                    else:
                        nc.vector.tensor_copy(
                            xT[:, dj, si * P : (si + 1) * P], tp[:]
                        )

        with (
            tc.tile_pool(name="wv", bufs=1) as wv,
            tc.tile_pool(name="wvstage", bufs=2) as wvstage,
            tc.tile_pool(name="wqk", bufs=2) as wqk_pool,
            tc.tile_pool(name="expp", bufs=5) as expp,
            tc.tile_pool(name="unp", bufs=2) as unp,
            tc.tile_pool(name="bcp", bufs=2) as bcp,
        ):
            # ---------------- B0: Q^T/K^T for pair 0 ----------------
            # emitted before C: B0's PE work runs while the V weights
            # stream in, and its wqk DMAs beat wv in the queue
            project_qk_tile(0, wqk_pool)
            project_qk_tile(PAIRS, wqk_pool)

            # V weights streamed per d-tile (full-D rows: 4 KB descriptor
            # rows; narrower slices halve effective DMA bandwidth).
            # Phase C runs in two dk-half contractions so it starts after
            # only half of W_v has landed (DMA is ~160 GB/s with 8 cores
            # sharing HBM); the dk0-3 partial is staged in bf16 v_part
            # (with the V bias folded in) and added in the second half.
            wv_tile = wv.tile([P, DT, D], bf16, tag="wv")
            v_part = wv.tile([P, ST, H, DK], bf16, tag="vpart")
            for half in range(2):
                for dk in range(4 * half, 4 * half + 4):
                    wv_stage = wvstage.tile([P, D], f32, tag="wvs")
                    nc.sync.dma_start(
                        wv_stage[:], w_in_kp[:, dk, 2 * D : 3 * D]
                    )
                    nc.vector.tensor_copy(wv_tile[:, dk, :], wv_stage[:])
                for st in range(ST):
                    ps = psum.tile([P, D], f32, tag="w", name="psc")
                    for dk in range(4 * half, 4 * half + 4):
                        nc.tensor.matmul(
                            ps[:, 0:512],
                            xT[:, dk, st * P : (st + 1) * P],
                            wv_tile[:, dk, 0:512],
                            start=(dk == 4 * half),
                            stop=(dk == 4 * half + 3),
                        )
                        nc.tensor.matmul(
                            ps[:, 512:1024],
                            xT[:, dk, st * P : (st + 1) * P],
                            wv_tile[:, dk, 512:1024],
                            start=(dk == 4 * half),
                            stop=(dk == 4 * half + 3),
                        )
                    if half == 0:
                        nc.vector.tensor_tensor(
                            v_part[:, st],
                            ps[:].rearrange("p (h d) -> p h d", d=DK),
                            bv_bc[:].rearrange("p (h d) -> p h d", d=DK),
                            ADD,
                        )
                    else:
                        nc.vector.tensor_tensor(
                            v_aug[:, st, :, 0:DK],
                            ps[:].rearrange("p (h d) -> p h d", d=DK),
                            v_part[:, st],
                            ADD,
                        )

            # ---------------- BD: attention ∥ next-pair projection ---------
            def emit_scores(pr, sc, sk):
                sps = psum.tile([P, S], f32, tag="w", name="sps")
                for hh in range(2):
                    base = hh * DK
                    nc.tensor.matmul(
                        sps[:, hh * 512 : (hh + 1) * 512],
                        qkT[
                            base : base + DK,
                            PAIRS + pr,
                            sk * P : (sk + 1) * P,
                        ],
                        qkT[base : base + DK, pr, sc * 512 : (sc + 1) * 512],
                        start=True,
                        stop=True,
                    )
                ex = expp.tile([P, S], bf16, tag="ex")
                nc.scalar.activation(ex[:], sps[:], EXP, scale=1.0 / np.sqrt(DK))
                return ex

            SBS = [(pr, sc) for pr in range(PAIRS) for sc in range(SC)]
            carry_ex0 = None
            w_next = None
            for idx, (pr, sc) in enumerate(SBS):
                # stage next pair's Q/K weights (DMA+cast run a pair ahead)
                if sc == 0 and pr + 1 < PAIRS:
                    w_next = [
                        stage_qk(pr + 1, wqk_pool),
                        stage_qk(PAIRS + pr + 1, wqk_pool),
                    ]
                nt_next = (pr + 1) if sc == 0 else (PAIRS + pr + 1)
                pv = [
                    pvps.tile([P, 512], f32, tag="pv", name=f"pv{i}")
                    for i in range(2)
                ]
                exps = {}
                # software pipeline: paired scores(sk) on PE, one wide
                # exp(sk) on ACT (hh halves share the tile), pv(sk-1);
                # next pair's projection chunks at sk 4/6 fill the PE
                # slack of this ACT-bound loop. The first scores of the
                # NEXT sub-block are emitted before PV(7) so ACT never
                # waits on the exp->PV(7)->scores serial tail.
                if carry_ex0 is not None:
                    exps[0] = carry_ex0
                    carry_ex0 = None
                else:
                    exps[0] = emit_scores(pr, sc, 0)
                for sk in range(1, ST + 1):
                    if sk < ST:
                        exps[sk] = emit_scores(pr, sc, sk)
                    if sk == ST - 1 and idx + 1 < len(SBS):
                        # pre-emit next sub-block's scores(0)+exp
                        npr, nsc = SBS[idx + 1]
                        carry_ex0 = emit_scores(npr, nsc, 0)
                    ex = exps.pop(sk - 1)
                    for hh in range(2):
                        h = 2 * pr + hh
                        nc.tensor.matmul(
                            pv[hh][0 : DK + 1, :],
                            v_aug[:, sk - 1, h, :],
                            ex[:, hh * 512 : (hh + 1) * 512],
                            start=(sk - 1 == 0),
                            stop=(sk - 1 == ST - 1),
                        )
                    if pr + 1 < PAIRS and sk in (4, 6):
                        project_chunk(
                            nt_next,
                            w_next[sc],
                            None,
                            0 if sk == 4 else 1,
                        )
                    # normalize. un copies emitted first: they are the only
                    # PSUM readers, so the pv slots release ~0.7/1.4us
                    # after stop regardless of the rest of the chain. Denom
                    # row to partition 0 (plain DVE copy does the p64->p0
                    # move; both the custom-ucode fast reciprocal and the
                    # GpSimd broadcast need partition-0 inputs). The final
                    # multiply runs on GpSimd, keeping DVE under ~60%.
                    uns, dns, rrs, bcs = [], [], [], []
                    for hh in range(2):
                        un = unp.tile([DK + 1, 512], f32, tag="un")
                        nc.vector.tensor_copy(un[:], pv[hh][0 : DK + 1, :])
                        uns.append(un)
                    for hh in range(2):
                        dn = bcp.tile([1, 512], f32, tag=f"dnrr{hh}")
                        nc.vector.tensor_copy(dn[:], uns[hh][DK : DK + 1, :])
                        dns.append(dn)
                    for hh in range(2):
                        rrow = bcp.tile([1, 512], f32, tag=f"dnrr{hh}")
                        nc.vector.reciprocal_approx_fast(rrow[:], dns[hh][:])
                        rrs.append(rrow)
                    for hh in range(2):
                        # full-tile broadcast (sliced outputs break on HW)
                        bc = bcp.tile([P, 512], f32, tag="bc")
                        nc.gpsimd.partition_broadcast(bc[:], rrs[hh][:])
                        bcs.append(bc)
                    for hh in range(2):
                        base = hh * DK
                        nc.vector.tensor_tensor(
                            attnT[
                                base : base + DK, pr, sc * 512 : (sc + 1) * 512
                            ],
                            uns[hh][0:DK, :],
                            bcs[hh][0:DK, :],
                            MULT,
                        )
                    # W_out prefetch + bf16 cast mid-loop (one d-tile per
                    # sub-block through the wv stage pool): after the front
                    # DMA burst drains, well before phase E needs it
                    if pr >= 2 and pr <= 5:
                        dk = 2 * (pr - 2) + sc
                        wo_stage = wvstage.tile([P, D], f32, tag="wvs")
                        nc.sync.dma_start(wo_stage[:], w_out_kp[:, dk, :])
                        nc.vector.tensor_copy(wout[:, dk, :], wo_stage[:])

        # ---------------- phase E: output projection (f32r) ----------------
        with tc.tile_pool(name="ypool", bufs=3) as ypool:
            # b_out broadcast staged here (GpSimd is idle during E);
            # keeps 4 KB/partition free during the BD loop
            bout_bc = ypool.tile([P, D], f32, tag="bout")
            bout_row = ypool.tile([1, D], f32, tag="boutrow")
            nc.sync.dma_start(bout_row[:], b_out[None, :])
            nc.gpsimd.partition_broadcast(bout_bc[:], bout_row[:])
            for st in range(ST):
                ps = psum.tile([P, D], f32, tag="w", name="pse")
                for ncx in range(SC):
                    for dk in range(DT):
                        nc.tensor.matmul(
                            ps[:, ncx * 512 : (ncx + 1) * 512],
                            attnT[:, dk, st * P : (st + 1) * P],
                            wout[:, dk, ncx * 512 : (ncx + 1) * 512],
                            start=(dk == 0),
                            stop=(dk == DT - 1),
                        )
                y = ypool.tile([P, D], f32, tag="y")
                nc.vector.tensor_tensor(y[:], ps[:], bout_bc[:], ADD)
                nc.sync.dma_start(out[st * P : (st + 1) * P, :], y[:])

        for pool in (
            wout_pool,
            attnT_pool,
            xT_pool,
            vaug_pool,
            qkT_pool,
            pvps,
            psum,
            const,
        ):
            pool.release()

    nc.finalize()
    return nc


_NC_CACHE = {}


def get_nc():
    if "nc" not in _NC_CACHE:
        _NC_CACHE["nc"] = build_nc()
    return _NC_CACHE["nc"]


def kernel(X, W_in, b_in, W_out, b_out):
    X = np.ascontiguousarray(np.asarray(X, dtype=np.float32))
    W_in = np.ascontiguousarray(np.asarray(W_in, dtype=np.float32))
    b_in = np.ascontiguousarray(np.asarray(b_in, dtype=np.float32))
    W_out = np.ascontiguousarray(np.asarray(W_out, dtype=np.float32))
    b_out = np.ascontiguousarray(np.asarray(b_out, dtype=np.float32))

    nc = get_nc()
    in_maps = [
        {"X": X[i], "W_in": W_in, "b_in": b_in, "W_out": W_out, "b_out": b_out}
        for i in range(B)
    ]
    res = run_bass_kernel_spmd(nc, in_maps, core_ids=list(range(B)))
    return np.stack([res.results[i]["out"] for i in range(B)], axis=0)
